# revision 23
# baseline (speedup 1.0000x reference)
"""MoE linear (modality-routed) Trainium2 kernel.

out[n] = x[n] @ W[modality_ids[n]].T + b[modality_ids[n]]

Strategy (data parallel over 8 cores, weight replicated):
- Host: per core shard of 16384 tokens, stable-argsort tokens by expert.
  Groups padded to a shared per-expert capacity (multiple of 128) so one
  SPMD NEFF serves all cores; per-tile expert is a compile-time constant.
- Device: x is host-cast to bf16 (halves gather traffic; W/PSUM/bias
  stay f32, total rel err ~3e-3 vs 2e-2 budget). Input side uses batched
  dma_gather (one Pool instruction per G=8 128-token tiles, int16
  indices wrap-16 across partitions) which amortizes the 994ns SWDGE
  fixed overhead; per tile: PE transpose
  (contraction dim to partitions) -> copy to SBUF on the Activation
  engine -> 4 accumulating fp32r matmuls against SBUF-resident W^T ->
  bias add on DVE (cast to bf16) -> per-tile indirect-DMA scatter of
  the bf16 row to the token's original row (host upcasts y to f32 while
  unsharding). The scatter's nominal out AP is a 128-row window;
  actual rows are selected by the index vector (base + idx*row_stride),
  padding slots point past bounds_check and are dropped.
"""

import sys

if "/opt/trn_rl_repo" not in sys.path:
    sys.path.insert(0, "/opt/trn_rl_repo")

import numpy as np
from ml_dtypes import bfloat16

import concourse.bass as bass  # noqa: F401
import concourse.tile as tile
from concourse import bacc, mybir
from concourse.bass import IndirectOffsetOnAxis
from concourse.bass_utils import run_bass_kernel_spmd
from concourse.masks import make_identity

N_CORES = 8
N_TOKENS = 131072
N_SHARD = N_TOKENS // N_CORES  # 16384
D_IN = 512
D_OUT = 512
N_EXPERTS = 3
P = 128
KC = D_IN // P  # 4 contraction chunks
G = 8  # tiles per dma_gather batch

_NC_CACHE = {}


def build_nc(n_shard, caps, num_devices=N_CORES):
    """Build + compile the SPMD Bass kernel for given per-expert capacities."""
    key = (n_shard, tuple(caps), num_devices)
    if key in _NC_CACHE:
        return _NC_CACHE[key]
    npad = sum(caps)
    nt = npad // P
    experts_of_tile = []
    for e, c in enumerate(caps):
        experts_of_tile += [e] * (c // P)

    nc = bacc.Bacc(
        "TRN2", target_bir_lowering=False, debug=False, num_devices=num_devices
    )
    f32 = mybir.dt.float32
    f32r = mybir.dt.float32r
    bf16 = mybir.dt.bfloat16
    i16 = mybir.dt.int16
    i32 = mybir.dt.int32

    # x is fed as bf16 (host-cast): halves gather traffic and speeds the PE
    # transposes; W/PSUM/bias/y stay f32.
    x = nc.dram_tensor("x", [n_shard, D_IN], bf16, kind="ExternalInput").ap()
    wt = nc.dram_tensor(
        "wt", [D_IN, N_EXPERTS * D_OUT], f32r, kind="ExternalInput"
    ).ap()
    bb = nc.dram_tensor(
        "bias_bc", [P, N_EXPERTS * D_OUT], f32, kind="ExternalInput"
    ).ap()
    gidx = nc.dram_tensor("gidx", [P, npad // 16], i16, kind="ExternalInput").ap()
    gdst = nc.dram_tensor("gdst", [P, npad // 16], i16, kind="ExternalInput").ap()
    zrs = nc.dram_tensor("zrs", [P, 4096], bf16, kind="ExternalInput").ap()
    y = nc.dram_tensor("y", [n_shard, D_OUT], bf16, kind="ExternalOutput").ap()

    with tile.TileContext(nc) as tc:
        with (
            tc.tile_pool(name="const", bufs=1) as cpool,
            tc.tile_pool(name="xg", bufs=6) as xg_pool,
            tc.tile_pool(name="xt", bufs=4) as xt_pool,
            tc.tile_pool(name="outp", bufs=4) as out_pool,
            tc.tile_pool(name="ptr", bufs=4, space="PSUM") as ptr_pool,
            tc.tile_pool(name="pmm", bufs=4, space="PSUM") as pmm_pool,
        ):
            ident = cpool.tile([P, P], bf16)
            make_identity(nc, ident[:])

            # Routing tables first: they gate the first gather/scatter,
            # while the (larger) weight loads are only needed by the first
            # matmul, ~10us later.
            gidx_sb = cpool.tile([P, npad // 16], i16)
            nc.sync.dma_start(out=gidx_sb[:], in_=gidx[:])
            gdst_sb = cpool.tile([P, npad // 16], i16)
            nc.sync.dma_start(out=gdst_sb[:], in_=gdst[:])

            # Zero y (scatter-add target): 16 chunks, a strided fence read
            # touching one row per chunk, then 4 fake-fence writes (512 zero
            # bytes onto already-zero row 0) tracked in the scatter-adds'
            # fake out-of-tensor windows — orders all zeroing before every
            # scatter-add without serializing the scatter-adds.
            ZC = n_shard // 1024
            zt = cpool.tile([P, 4096], bf16)
            nc.sync.dma_start(out=zt[:], in_=zrs[:])
            for c in range(ZC):
                nc.sync.dma_start(out=y[c * 1024 : (c + 1) * 1024, :], in_=zt[:])
            fence = cpool.tile([ZC, D_OUT], bf16)
            nc.sync.dma_start(out=fence[:], in_=y[0 : n_shard : 1024, :])
            for w in range(4):
                row0 = y[0:1]
                row0 = bass.AP(
                    tensor=row0.tensor,
                    offset=row0.offset,
                    ap=row0.ap,
                    dep_tracking_offset=(w + 1) * n_shard * D_OUT,
                )
                nc.sync.dma_start(out=row0, in_=fence[0:1, :])
            bias_sb = cpool.tile([P, N_EXPERTS * D_OUT], f32)
            nc.sync.dma_start(out=bias_sb[:], in_=bb[:])
            # W^T resident in SBUF: block (e, kc) is [k=128, o=512]
            w_sb = cpool.tile([P, N_EXPERTS * KC * D_OUT], f32r)
            for e in range(N_EXPERTS):
                for kc in range(KC):
                    nc.sync.dma_start(
                        out=w_sb[:, (e * KC + kc) * D_OUT : (e * KC + kc + 1) * D_OUT],
                        in_=wt[kc * P : (kc + 1) * P, e * D_OUT : (e + 1) * D_OUT],
                    )

            # Two software pipelines over program order:
            # - matmuls are emitted MM_DELAY tiles behind transposes, so the
            #   PE sequencer (head-of-line) never waits on the Act-engine
            #   PSUM->SBUF copy; the wait is hidden under later transposes.
            # - the batched scatter-add for batch b is emitted one batch
            #   later, so the Pool sequencer never stalls on an unfinished
            #   bias add. num_idxs registers are created inline per op (the
            #   Q7 custom-DMA ucode is not safe with long-lived registers).
            MM_DELAY = 2
            SC_DELAY = 1
            mm_pending = []
            sc_pending = []

            def emit_matmul(t, xt, osb, j):
                e = experts_of_tile[t]
                pmm = pmm_pool.tile([P, D_OUT], f32)
                for kc in range(KC):
                    nc.tensor.matmul(
                        pmm[:],
                        lhsT=xt[:, kc * P : (kc + 1) * P],
                        rhs=w_sb[:, (e * KC + kc) * D_OUT : (e * KC + kc + 1) * D_OUT],
                        start=(kc == 0),
                        stop=(kc == KC - 1),
                    )
                nc.vector.tensor_add(
                    out=osb[:, j, :],
                    in0=pmm[:],
                    in1=bias_sb[:, e * D_OUT : (e + 1) * D_OUT],
                )

            def emit_scatter(b, g, osb):
                # One scatter-add for g tiles: osb[p, j, :] += to y row
                # gdst[j*128+p]; padding carries negative indices (dropped by
                # the ucode); every real row is written exactly once onto
                # zeros, so += is assignment. Disjoint fake out-of-tensor
                # dep-tracking windows stop consecutive scatter-adds from
                # WAW-chaining while ordering each behind its fake fence.
                win = y[:]
                win = bass.AP(
                    tensor=win.tensor,
                    offset=win.offset,
                    ap=win.ap,
                    dep_tracking_offset=(b % 4 + 1) * n_shard * D_OUT,
                )
                nc.gpsimd.dma_scatter_add(
                    win,
                    osb[:],
                    gdst_sb[:, b * G * (P // 16) : (b * G + g) * (P // 16)],
                    g * P,
                    g * P,
                    D_OUT,
                )

            for t0 in range(0, nt, G):
                g = min(G, nt - t0)
                # Batched gather: xg[p, j, :] = x[idxs[j*128+p]] where idxs
                # covers sorted slots [t0*128, (t0+g)*128).
                xg = xg_pool.tile([P, g, D_IN], bf16)
                nc.gpsimd.dma_gather(
                    xg[:],
                    x[:],
                    gidx_sb[:, t0 * (P // 16) : (t0 + g) * (P // 16)],
                    g * P,
                    g * P,
                    D_IN,
                )
                osb = out_pool.tile([P, g, D_OUT], bf16)
                for j in range(g):
                    t = t0 + j
                    ptr = ptr_pool.tile([P, D_IN], bf16)
                    for kc in range(KC):
                        nc.tensor.transpose(
                            ptr[:, kc * P : (kc + 1) * P],
                            xg[:, j, kc * P : (kc + 1) * P],
                            ident[:],
                        )
                    xt = xt_pool.tile([P, D_IN], f32r)
                    nc.scalar.copy(xt[:], ptr[:])
                    mm_pending.append((t, xt, osb, j))
                    if len(mm_pending) > MM_DELAY:
                        emit_matmul(*mm_pending.pop(0))
                sc_pending.append((t0 // G, g, osb))
                if len(sc_pending) > SC_DELAY:
                    emit_scatter(*sc_pending.pop(0))
            for t, xt, osb, j in mm_pending:
                emit_matmul(t, xt, osb, j)
            for b, g, osb in sc_pending:
                emit_scatter(b, g, osb)

    nc.compile()
    _NC_CACHE[key] = nc
    return nc


def make_routing(ids_shard, caps):
    """Per-core routing tables.

    gidx [P, npad//16] int16: dma_gather indices, wrap-16 per G-tile batch,
    replicated on 8x16 partitions. Padding slots gather row 0 (dropped later).
    gdst [P, npad//16] int16: scatter-add destinations, wrap-16 per batch;
    padding -> -1 (dropped by the ucode).
    """
    n_shard = ids_shard.shape[0]
    npad = sum(caps)
    nt = npad // P
    order = np.argsort(ids_shard, kind="stable").astype(np.int64)
    cnt = np.bincount(ids_shard, minlength=N_EXPERTS)
    gs = np.zeros(npad, np.int64)
    gd = np.full(npad, -1, np.int64)
    base = 0
    off = 0
    for e in range(N_EXPERTS):
        c = int(cnt[e])
        seg = order[off : off + c]
        gs[base : base + c] = seg
        gd[base : base + c] = seg
        base += caps[e]
        off += c
    def wrap16(arr):
        blocks = []
        for t0 in range(0, nt, G):
            g = min(G, nt - t0)
            blk = arr[t0 * P : (t0 + g) * P]
            blocks.append(np.ascontiguousarray(blk.reshape(-1, 16).T))
        return np.tile(np.concatenate(blocks, axis=1), (8, 1)).astype(np.int16)

    return wrap16(gs), wrap16(gd)


def prepare(inputs):
    """Shared host-side prep: returns (nc, in_maps)."""
    x = np.ascontiguousarray(np.asarray(inputs["x"], dtype=np.float32))
    ids = np.asarray(inputs["modality_ids"]).astype(np.int64)
    weight = np.asarray(inputs["weight"], dtype=np.float32)
    b = np.asarray(inputs["bias"], dtype=np.float32)

    wt = np.ascontiguousarray(weight.T)  # [D_IN, E*D_OUT]
    bias_bc = np.ascontiguousarray(
        np.broadcast_to(b[None, :], (P, N_EXPERTS * D_OUT))
    )

    counts = np.stack(
        [
            np.bincount(ids[c * N_SHARD : (c + 1) * N_SHARD], minlength=N_EXPERTS)
            for c in range(N_CORES)
        ]
    )
    caps = [int(-(-counts[:, e].max() // P) * P) for e in range(N_EXPERTS)]

    nc = build_nc(N_SHARD, caps)
    in_maps = []
    for c in range(N_CORES):
        ids_c = ids[c * N_SHARD : (c + 1) * N_SHARD]
        gidx, gdst = make_routing(ids_c, caps)
        in_maps.append(
            {
                "x": np.ascontiguousarray(
                    x[c * N_SHARD : (c + 1) * N_SHARD].astype(bfloat16)
                ),
                "wt": wt,
                "bias_bc": bias_bc,
                "gidx": gidx,
                "gdst": gdst,
                "zrs": np.zeros((P, 4096), dtype=bfloat16),
            }
        )
    return nc, in_maps


def run(inputs, trace=False):
    """Returns (out, BassKernelResults)."""
    nc, in_maps = prepare(inputs)
    res = run_bass_kernel_spmd(nc, in_maps, list(range(N_CORES)), trace=trace)
    out = np.concatenate(
        [res.results[c]["y"] for c in range(N_CORES)], axis=0
    ).astype(np.float32)  # bf16 -> f32 upcast during unshard
    return out, res


def kernel(**inputs):
    out, _ = run(inputs, trace=False)
    return out


# revision 27
# speedup vs baseline: 1.0375x; 1.0375x over previous
"""MoE linear (modality-routed) Trainium2 kernel.

out[n] = x[n] @ W[modality_ids[n]].T + b[modality_ids[n]]

Strategy (data parallel over 8 cores, weight replicated):
- Host: per core shard of 16384 tokens, stable-argsort tokens by expert.
  Groups padded to a shared per-expert capacity (multiple of 128) so one
  SPMD NEFF serves all cores; per-tile expert is a compile-time constant.
- Device: x is host-cast to bf16 (halves gather traffic; W/PSUM/bias
  stay f32, total rel err ~3e-3 vs 2e-2 budget). Input side uses batched
  dma_gather (one Pool instruction per G=8 128-token tiles, int16
  indices wrap-16 across partitions) which amortizes the 994ns SWDGE
  fixed overhead; per tile: PE transpose
  (contraction dim to partitions) -> copy to SBUF on the Activation
  engine -> 4 accumulating fp32r matmuls against SBUF-resident W^T ->
  bias add on DVE (cast to bf16) -> per-tile indirect-DMA scatter of
  the bf16 row to the token's original row (host upcasts y to f32 while
  unsharding). The scatter's nominal out AP is a 128-row window;
  actual rows are selected by the index vector (base + idx*row_stride),
  padding slots point past bounds_check and are dropped.
"""

import sys

if "/opt/trn_rl_repo" not in sys.path:
    sys.path.insert(0, "/opt/trn_rl_repo")

import numpy as np
from ml_dtypes import bfloat16

import concourse.bass as bass  # noqa: F401
import concourse.tile as tile
from concourse import bacc, mybir
from concourse.bass import IndirectOffsetOnAxis
from concourse.bass_utils import run_bass_kernel_spmd
from concourse.masks import make_identity

N_CORES = 8
N_TOKENS = 131072
N_SHARD = N_TOKENS // N_CORES  # 16384
D_IN = 512
D_OUT = 512
N_EXPERTS = 3
P = 128
KC = D_IN // P  # 4 contraction chunks
G = 8  # tiles per dma_gather batch

_NC_CACHE = {}


def build_nc(n_shard, caps, num_devices=N_CORES):
    """Build + compile the SPMD Bass kernel for given per-expert capacities."""
    key = (n_shard, tuple(caps), num_devices)
    if key in _NC_CACHE:
        return _NC_CACHE[key]
    npad = sum(caps)
    nt = npad // P
    experts_of_tile = []
    for e, c in enumerate(caps):
        experts_of_tile += [e] * (c // P)

    nc = bacc.Bacc(
        "TRN2", target_bir_lowering=False, debug=False, num_devices=num_devices
    )
    f32 = mybir.dt.float32
    f32r = mybir.dt.float32r
    bf16 = mybir.dt.bfloat16
    i16 = mybir.dt.int16
    i32 = mybir.dt.int32

    # x is fed as bf16 (host-cast): halves gather traffic and speeds the PE
    # transposes; W/PSUM/bias/y stay f32.
    x = nc.dram_tensor("x", [n_shard, D_IN], bf16, kind="ExternalInput").ap()
    wt = nc.dram_tensor(
        "wt", [D_IN, N_EXPERTS * D_OUT], f32r, kind="ExternalInput"
    ).ap()
    bb = nc.dram_tensor(
        "bias_bc", [P, N_EXPERTS * D_OUT], f32, kind="ExternalInput"
    ).ap()
    gidx = nc.dram_tensor("gidx", [P, npad // 16], i16, kind="ExternalInput").ap()
    gdst = nc.dram_tensor("gdst", [P, npad // 16], i16, kind="ExternalInput").ap()
    zrs = nc.dram_tensor("zrs", [P, 4096], bf16, kind="ExternalInput").ap()
    # One extra dump row at index n_shard: padding slots scatter-add there
    # (avoids the ucode's racy negative-index drop path); host discards it.
    y = nc.dram_tensor("y", [n_shard + 1, D_OUT], bf16, kind="ExternalOutput").ap()

    with tile.TileContext(nc) as tc:
        with (
            tc.tile_pool(name="const", bufs=1) as cpool,
            tc.tile_pool(name="xg", bufs=6) as xg_pool,
            tc.tile_pool(name="xt", bufs=4) as xt_pool,
            tc.tile_pool(name="outp", bufs=4) as out_pool,
            tc.tile_pool(name="ptr", bufs=4, space="PSUM") as ptr_pool,
            tc.tile_pool(name="pmm", bufs=4, space="PSUM") as pmm_pool,
        ):
            ident = cpool.tile([P, P], bf16)
            make_identity(nc, ident[:])

            # Routing tables first: they gate the first gather/scatter,
            # while the (larger) weight loads are only needed by the first
            # matmul, ~10us later.
            gidx_sb = cpool.tile([P, npad // 16], i16)
            nc.sync.dma_start(out=gidx_sb[:], in_=gidx[:])
            gdst_sb = cpool.tile([P, npad // 16], i16)
            nc.sync.dma_start(out=gdst_sb[:], in_=gdst[:])

            bias_sb = cpool.tile([P, N_EXPERTS * D_OUT], f32)
            nc.sync.dma_start(out=bias_sb[:], in_=bb[:])
            # W^T resident in SBUF: block (e, kc) is [k=128, o=512]
            w_sb = cpool.tile([P, N_EXPERTS * KC * D_OUT], f32r)
            for e in range(N_EXPERTS):
                for kc in range(KC):
                    nc.sync.dma_start(
                        out=w_sb[:, (e * KC + kc) * D_OUT : (e * KC + kc + 1) * D_OUT],
                        in_=wt[kc * P : (kc + 1) * P, e * D_OUT : (e + 1) * D_OUT],
                    )
            # Zero y (scatter-add target): 16 chunks, a strided fence read
            # touching one row per chunk, then 4 fake-fence writes (512 zero
            # bytes onto already-zero row 0) tracked in the scatter-adds'
            # fake out-of-tensor windows — orders all zeroing before every
            # scatter-add without serializing the scatter-adds.
            ZC = n_shard // 1024
            zt = cpool.tile([P, 4096], bf16)
            nc.scalar.dma_start(out=zt[:], in_=zrs[:])
            for c in range(ZC):
                nc.scalar.dma_start(out=y[c * 1024 : (c + 1) * 1024, :], in_=zt[:])
            fence = cpool.tile([ZC, D_OUT], bf16)
            nc.sync.dma_start(out=fence[:], in_=y[0 : n_shard : 1024, :])
            for w in range(4):
                row0 = y[0:1]
                row0 = bass.AP(
                    tensor=row0.tensor,
                    offset=row0.offset,
                    ap=row0.ap,
                    dep_tracking_offset=(w + 1) * n_shard * D_OUT,
                )
                nc.sync.dma_start(out=row0, in_=fence[0:1, :])

            # Two software pipelines over program order:
            # - matmuls are emitted MM_DELAY tiles behind transposes, so the
            #   PE sequencer (head-of-line) never waits on the Act-engine
            #   PSUM->SBUF copy; the wait is hidden under later transposes.
            # - the batched scatter-add for batch b is emitted one batch
            #   later, so the Pool sequencer never stalls on an unfinished
            #   bias add. num_idxs registers are created inline per op (the
            #   Q7 custom-DMA ucode is not safe with long-lived registers).
            MM_DELAY = 2
            SC_DELAY = 2
            mm_pending = []
            sc_pending = []

            def emit_matmul(t, xt, osb, j):
                e = experts_of_tile[t]
                pmm = pmm_pool.tile([P, D_OUT], f32)
                for kc in range(KC):
                    nc.tensor.matmul(
                        pmm[:],
                        lhsT=xt[:, kc * P : (kc + 1) * P],
                        rhs=w_sb[:, (e * KC + kc) * D_OUT : (e * KC + kc + 1) * D_OUT],
                        start=(kc == 0),
                        stop=(kc == KC - 1),
                    )
                nc.vector.tensor_add(
                    out=osb[:, j, :],
                    in0=pmm[:],
                    in1=bias_sb[:, e * D_OUT : (e + 1) * D_OUT],
                )

            def emit_scatter(b, g, osb):
                # One scatter-add for g tiles: osb[p, j, :] += to y row
                # gdst[j*128+p]; padding rows target the dump row n_shard;
                # every real row is written exactly once onto zeros, so +=
                # is assignment. Disjoint fake out-of-tensor
                # dep-tracking windows stop consecutive scatter-adds from
                # WAW-chaining while ordering each behind its fake fence.
                win = y[:]
                win = bass.AP(
                    tensor=win.tensor,
                    offset=win.offset,
                    ap=win.ap,
                    dep_tracking_offset=(b % 4 + 1) * n_shard * D_OUT,
                )
                nc.gpsimd.dma_scatter_add(
                    win,
                    osb[:],
                    gdst_sb[:, b * G * (P // 16) : (b * G + g) * (P // 16)],
                    g * P,
                    g * P,
                    D_OUT,
                )

            for t0 in range(0, nt, G):
                g = min(G, nt - t0)
                # Batched gather: xg[p, j, :] = x[idxs[j*128+p]] where idxs
                # covers sorted slots [t0*128, (t0+g)*128).
                xg = xg_pool.tile([P, g, D_IN], bf16)
                nc.gpsimd.dma_gather(
                    xg[:],
                    x[:],
                    gidx_sb[:, t0 * (P // 16) : (t0 + g) * (P // 16)],
                    g * P,
                    g * P,
                    D_IN,
                )
                osb = out_pool.tile([P, g, D_OUT], bf16)
                for j in range(g):
                    t = t0 + j
                    ptr = ptr_pool.tile([P, D_IN], bf16)
                    for kc in range(KC):
                        nc.tensor.transpose(
                            ptr[:, kc * P : (kc + 1) * P],
                            xg[:, j, kc * P : (kc + 1) * P],
                            ident[:],
                        )
                    xt = xt_pool.tile([P, D_IN], f32r)
                    nc.scalar.copy(xt[:], ptr[:])
                    mm_pending.append((t, xt, osb, j))
                    if len(mm_pending) > MM_DELAY:
                        emit_matmul(*mm_pending.pop(0))
                sc_pending.append((t0 // G, g, osb))
                if len(sc_pending) > SC_DELAY:
                    emit_scatter(*sc_pending.pop(0))
            for t, xt, osb, j in mm_pending:
                emit_matmul(t, xt, osb, j)
            for b, g, osb in sc_pending:
                emit_scatter(b, g, osb)

    nc.compile()
    _NC_CACHE[key] = nc
    return nc


def make_routing(ids_shard, caps):
    """Per-core routing tables.

    gidx [P, npad//16] int16: dma_gather indices, wrap-16 per G-tile batch,
    replicated on 8x16 partitions. Padding slots gather row 0 (dropped later).
    gdst [P, npad//16] int16: scatter-add destinations, wrap-16 per batch;
    padding -> n_shard (a dump row the host discards).
    """
    n_shard = ids_shard.shape[0]
    npad = sum(caps)
    nt = npad // P
    order = np.argsort(ids_shard, kind="stable").astype(np.int64)
    cnt = np.bincount(ids_shard, minlength=N_EXPERTS)
    gs = np.zeros(npad, np.int64)
    gd = np.full(npad, n_shard, np.int64)
    base = 0
    off = 0
    for e in range(N_EXPERTS):
        c = int(cnt[e])
        seg = order[off : off + c]
        gs[base : base + c] = seg
        gd[base : base + c] = seg
        base += caps[e]
        off += c
    def wrap16(arr):
        blocks = []
        for t0 in range(0, nt, G):
            g = min(G, nt - t0)
            blk = arr[t0 * P : (t0 + g) * P]
            blocks.append(np.ascontiguousarray(blk.reshape(-1, 16).T))
        return np.tile(np.concatenate(blocks, axis=1), (8, 1)).astype(np.int16)

    return wrap16(gs), wrap16(gd)


def prepare(inputs):
    """Shared host-side prep: returns (nc, in_maps)."""
    x = np.ascontiguousarray(np.asarray(inputs["x"], dtype=np.float32))
    ids = np.asarray(inputs["modality_ids"]).astype(np.int64)
    weight = np.asarray(inputs["weight"], dtype=np.float32)
    b = np.asarray(inputs["bias"], dtype=np.float32)

    wt = np.ascontiguousarray(weight.T)  # [D_IN, E*D_OUT]
    bias_bc = np.ascontiguousarray(
        np.broadcast_to(b[None, :], (P, N_EXPERTS * D_OUT))
    )

    counts = np.stack(
        [
            np.bincount(ids[c * N_SHARD : (c + 1) * N_SHARD], minlength=N_EXPERTS)
            for c in range(N_CORES)
        ]
    )
    caps = [int(-(-counts[:, e].max() // P) * P) for e in range(N_EXPERTS)]

    nc = build_nc(N_SHARD, caps)
    in_maps = []
    for c in range(N_CORES):
        ids_c = ids[c * N_SHARD : (c + 1) * N_SHARD]
        gidx, gdst = make_routing(ids_c, caps)
        in_maps.append(
            {
                "x": np.ascontiguousarray(
                    x[c * N_SHARD : (c + 1) * N_SHARD].astype(bfloat16)
                ),
                "wt": wt,
                "bias_bc": bias_bc,
                "gidx": gidx,
                "gdst": gdst,
                "zrs": np.zeros((P, 4096), dtype=bfloat16),
            }
        )
    return nc, in_maps


def run(inputs, trace=False):
    """Returns (out, BassKernelResults)."""
    nc, in_maps = prepare(inputs)
    res = run_bass_kernel_spmd(nc, in_maps, list(range(N_CORES)), trace=trace)
    out = np.concatenate(
        [res.results[c]["y"][:N_SHARD] for c in range(N_CORES)], axis=0
    ).astype(np.float32)  # bf16 -> f32 upcast during unshard; dump row dropped
    return out, res


def kernel(**inputs):
    out, _ = run(inputs, trace=False)
    return out
